# revision 12
# baseline (speedup 1.0000x reference)
# Circular conv along channels (len 2048, first 1536 outputs kept), computed
# as a BLOCK-DIAGONAL matmul in a partially-FFT'd basis: recursively factor
# z^2048-1 over R down to dense real blocks of size 128 (cyc -> cyc+nega;
# nega -> complex twisted; twisted -> twist pair). The host applies the O(n)
# butterflies per row; each core multiplies its 4096-row shard by sixteen
# dense 128x128 fp16 blocks and streams results back.
#
# I/O rides int8 both ways (HBM traffic 17.3MB/core vs 33.5MB for fp16):
# the host quantizes the transform coords to int8 (clip 4.2 sigma); the
# device upcasts int8->fp16 (exact) on DVE/GpSimd, matmuls in fp16 against
# weights that fold the input step and per-output-column scales, and evicts
# PSUM directly to int8 (HW cast = round-to-nearest-even + saturate, probed)
# on ACT/DVE. Host decodes with per-column scales and inverts the CRT
# combines. Quantization noise ~1.4e-2 rel (budget 2e-2), measured in sim.
import numpy as np

IN_DIM = 2048
OUT_DIM = 1536
N_CORES = 8
ROWS = 8 * 64 * 64            # 32768
RPC = ROWS // N_CORES         # 4096 rows per core

P = 128                       # partitions
BETA = 128                    # dense block real size
NB = IN_DIM // BETA           # 16 diagonal blocks
K_TILES = IN_DIM // P         # 16 (== NB at BETA=128)
BLK = 512                     # matmul N (one PSUM bank fp32)
RQ = RPC // BLK               # 8 row-quarters per block
CLIP = 4.2                    # quantization clip, in sigmas

_cache = {}


# ---------- recursive CRT factorization (host side, numpy) ----------

def _fwd_x(x):
    """x [N, 2048] f32 -> X' [N, 2048] block inputs (real f32)."""

    def rec(kind, arr, theta):
        if kind == "cyc":
            n = arr.shape[1]
            if n <= BETA:
                return [arr]
            lo, hi = arr[:, :n // 2], arr[:, n // 2:]
            return rec("cyc", lo + hi, None) + rec("nega", lo - hi, None)
        if kind == "nega":
            n = arr.shape[1]
            if n <= BETA:
                return [arr]
            A = arr[:, :n // 2] + 1j * arr[:, n // 2:]
            return rec("tw", A, 1j)
        m = arr.shape[1]
        if 2 * m <= BETA:
            return [np.concatenate([arr.real, arr.imag], axis=1)]
        s = np.sqrt(theta)
        lo, hi = arr[:, :m // 2], arr[:, m // 2:]
        return rec("tw", lo + s * hi, s) + rec("tw", lo - s * hi, -s)

    return np.concatenate(rec("cyc", x, None), axis=1)


def _build_mats(w):
    """w [2048] f64 -> list of dense real block matrices (sum sizes = 2048)."""

    def twisted(Bp, theta):
        m = len(Bp)
        k = np.arange(m)
        idx = (k[None, :] - k[:, None]) % m
        wrap = k[None, :] < k[:, None]
        return Bp[idx] * np.where(wrap, theta, 1.0)

    def rec(kind, arr, theta):
        if kind == "cyc":
            n = len(arr)
            if n <= BETA:
                k = np.arange(n)
                return [arr[(k[None, :] - k[:, None]) % n]]
            lo, hi = arr[:n // 2], arr[n // 2:]
            return rec("cyc", lo + hi, None) + rec("nega", lo - hi, None)
        if kind == "nega":
            n = len(arr)
            if n <= BETA:
                k = np.arange(n)
                sgn = np.where(k[None, :] >= k[:, None], 1.0, -1.0)
                return [arr[(k[None, :] - k[:, None]) % n] * sgn]
            A = arr[:n // 2] + 1j * arr[n // 2:]
            return rec("tw", A, 1j)
        m = len(arr)
        if 2 * m <= BETA:
            T = twisted(arr, theta)
            return [np.block([[T.real, T.imag], [-T.imag, T.real]])]
        s = np.sqrt(theta)
        lo, hi = arr[:m // 2], arr[m // 2:]
        return rec("tw", lo + s * hi, s) + rec("tw", lo - s * hi, -s)

    return rec("cyc", w, None)


def _inv_y(Yp):
    """block outputs Y' [N, 2048] f32 -> y [N, 1536] f32."""

    def rec(kind, n_real, theta, cols):
        if kind == "cyc":
            if n_real <= BETA:
                return cols.pop(0)
            a = rec("cyc", n_real // 2, None, cols)
            b = rec("nega", n_real // 2, None, cols)
            return np.concatenate([(a + b) * 0.5, (a - b) * 0.5], axis=1)
        if kind == "nega":
            if n_real <= BETA:
                return cols.pop(0)
            Cc = rec("tw", n_real // 2, 1j, cols)
            return np.concatenate([Cc.real, Cc.imag], axis=1)
        m = n_real
        if 2 * m <= BETA:
            blk = cols.pop(0)
            return blk[:, :m] + 1j * blk[:, m:]
        s = np.sqrt(theta)
        U = rec("tw", m // 2, s, cols)
        V = rec("tw", m // 2, -s, cols)
        return np.concatenate([(U + V) * 0.5, (U - V) * (0.5 / s)], axis=1)

    def widths(kind, n, out):
        if kind == "cyc":
            if n <= BETA:
                out.append(n)
                return
            widths("cyc", n // 2, out)
            widths("nega", n // 2, out)
            return
        if kind == "nega":
            if n <= BETA:
                out.append(n)
                return
            widths("tw", n // 2, out)
            return
        if 2 * n <= BETA:
            out.append(2 * n)
            return
        widths("tw", n // 2, out)
        widths("tw", n // 2, out)

    ws = []
    widths("cyc", IN_DIM, ws)
    cols, off = [], 0
    for w_real in ws:
        cols.append(Yp[:, off:off + w_real])
        off += w_real
    y = rec("cyc", IN_DIM, None, cols)
    return y[:, :OUT_DIM]


def build_blocks(W_first_col, W_second_col, sig_x):
    """-> (mm [128, 16, 128] fp16 with folded scales, s_out [2048] f32).

    mm[p, b, :] = row p of scaled block b. s_out decodes int8 outputs."""
    w = (np.asarray(W_first_col, np.float64)
         * np.asarray(W_second_col, np.float64))[:IN_DIM]
    mats = _build_mats(w)
    assert all(M.shape == (BETA, BETA) for M in mats), [M.shape for M in mats]
    d_in = CLIP * sig_x / 127.0
    coln = np.concatenate([np.linalg.norm(M, axis=0) for M in mats])  # [2048]
    s_out = (CLIP / 127.0) * coln * sig_x                             # [2048]
    mm3d = np.stack(mats, axis=0)                                     # [NB, B, B]
    mm3d = d_in * mm3d / s_out.reshape(NB, 1, BETA)
    mm2d = mm3d.reshape(IN_DIM, BETA)
    return (np.ascontiguousarray(
        mm2d.reshape(K_TILES, P, BETA).transpose(1, 0, 2)).astype(np.float16),
        s_out.astype(np.float32))


def quant_rows(x2d):
    """x [N, 2048] f32 -> (X8 [N, 2048] int8, sig_x)."""
    Xp = _fwd_x(np.asarray(x2d, np.float32))
    sig_x = float(Xp.std())
    d_in = CLIP * sig_x / 127.0
    X8 = np.clip(np.rint(Xp * (1.0 / d_in)), -127, 127).astype(np.int8)
    return X8, sig_x


def shard_xT(X8, c):
    """core shard -> xT8 [P, K_TILES, RPC] int8: xT8[p,j,r] = X8[cR+r, 128j+p]."""
    sh = X8[c * RPC:(c + 1) * RPC]                   # [RPC, 2048] int8
    return np.ascontiguousarray(
        sh.reshape(RPC, K_TILES, P).transpose(2, 1, 0))


# ---------- device kernel ----------

def _emit_body(nc, xts, xfpool, otpool, pspool, wt, yT8):
    import concourse.mybir as mybir

    for og in range(NB):
        xf = xfpool.tile([P, RPC], mybir.dt.float16, name=f"xf{og}", tag="xf")
        # upcast int8 -> fp16 (exact) on DVE (measured 1.7us/block)
        nc.vector.tensor_copy(xf[:], xts[og // 2][:, og % 2, :])

        lhsT = wt[:, og, :]
        ot = otpool.tile([P, RPC], mybir.dt.int8, name=f"o{og}", tag="ot")
        # DVE takes 2 pair-evictions on two ogs, 1 elsewhere; ACT the rest
        n_dve = 2 if og in (5, 11) else 1
        for pr in range(RQ // 2):
            ps = pspool.tile([P, 2, BLK], mybir.dt.float32,
                             name=f"ps{og}_{pr}", tag=f"ps{pr}")
            for j in range(2):
                rq = pr * 2 + j
                nc.tensor.matmul(ps[:, j, :], lhsT,
                                 xf[:, rq * BLK:(rq + 1) * BLK],
                                 start=True, stop=True)
            # evict PSUM -> int8 (HW cast = round-to-nearest + saturate)
            dst = ot[:, pr * 2 * BLK:(pr + 1) * 2 * BLK]
            if pr >= (RQ // 2) - n_dve:
                nc.vector.tensor_copy(dst, ps[:, :, :])
            else:
                nc.scalar.copy(dst, ps[:, :, :])
        # out-DMAs ride the ACT HWDGE ring; input keeps the SP ring
        nc.gpsimd.dma_start(yT8[og * P:(og + 1) * P, :], ot[:])


def _build(repeat=1):
    import concourse.bass as bass
    import concourse.mybir as mybir
    import concourse.tile as tile
    from concourse import bacc

    nc = bacc.Bacc(
        "TRN2",
        target_bir_lowering=False,
        debug=False,
        enable_asserts=False,
        num_devices=N_CORES,
    )
    xT8 = nc.dram_tensor("xT8", (P, K_TILES, RPC), mybir.dt.int8,
                         kind="ExternalInput")
    mm = nc.dram_tensor("mm", (P, K_TILES, BETA), mybir.dt.float16,
                        kind="ExternalInput")
    yT8 = nc.dram_tensor("yT8", (IN_DIM, RPC), mybir.dt.int8,
                         kind="ExternalOutput")

    with tile.TileContext(nc) as tc:
        with (
            tc.tile_pool(name="w", bufs=1) as wpool,
            tc.tile_pool(name="x", bufs=1) as xpool,
            tc.tile_pool(name="xf", bufs=3) as xfpool,
            tc.tile_pool(name="o", bufs=3) as otpool,
            tc.tile_pool(name="ps", bufs=1, space=bass.MemorySpace.PSUM) as pspool,
        ):
            # weights ride the (initially idle) ACT ring, off the input path
            wt = wpool.tile([P, K_TILES, BETA], mybir.dt.float16, name="wt")
            nc.scalar.dma_start(wt[:], mm[:])

            # single-block first loads so og0 starts ~1.5us earlier
            xts = []
            for q in range(NB // 2):
                t = xpool.tile([P, 2, RPC], mybir.dt.int8,
                               name=f"x{q}", tag=f"x{q}")
                if q == 0:
                    nc.sync.dma_start(t[:, 0, :], xT8[:, 0, :])
                    nc.sync.dma_start(t[:, 1, :], xT8[:, 1, :])
                else:
                    nc.sync.dma_start(t[:], xT8[:, 2 * q:2 * q + 2, :])
                xts.append(t)

            if repeat > 1:
                with tc.For_i(0, repeat, 1):
                    _emit_body(nc, xts, xfpool, otpool, pspool, wt, yT8)
            else:
                _emit_body(nc, xts, xfpool, otpool, pspool, wt, yT8)

    nc.compile()
    return nc


def kernel(x: np.ndarray, W_first_col: np.ndarray, W_second_col: np.ndarray) -> np.ndarray:
    from concourse import bass_utils

    X8, sig_x = quant_rows(np.asarray(x, np.float32).reshape(ROWS, IN_DIM))
    mmat, s_out = build_blocks(W_first_col, W_second_col, sig_x)
    in_maps = [{"xT8": shard_xT(X8, c), "mm": mmat} for c in range(N_CORES)]

    if "nc" not in _cache:
        _cache["nc"] = _build()
    try:
        res = bass_utils.run_bass_kernel_spmd(
            _cache["nc"], in_maps, core_ids=list(range(N_CORES))
        )
    except Exception:
        # transient device/exec failures usually clear on a retry
        res = bass_utils.run_bass_kernel_spmd(
            _cache["nc"], in_maps, core_ids=list(range(N_CORES))
        )
    Yq = np.concatenate(
        [np.ascontiguousarray(r["yT8"].T) for r in res.results], axis=0)
    out = _inv_y(Yq.astype(np.float32) * s_out[None, :])
    return np.ascontiguousarray(out.astype(np.float32)).reshape(
        8, 64, 64, OUT_DIM)


# revision 19
# speedup vs baseline: 1.0258x; 1.0258x over previous
# Circular conv along channels (len 2048, first 1536 outputs kept), computed
# as a BLOCK-DIAGONAL matmul in a partially-FFT'd basis: recursively factor
# z^2048-1 over R down to dense real blocks of size 128 (cyc -> cyc+nega;
# nega -> complex twisted; twisted -> twist pair). The host applies the O(n)
# butterflies per row; each core multiplies its 4096-row shard by sixteen
# dense 128x128 fp16 blocks and streams results back.
#
# I/O rides int8 both ways (HBM traffic 17.3MB/core vs 33.5MB for fp16):
# the host quantizes the transform coords to int8 (clip 4.2 sigma); the
# device upcasts int8->fp16 (exact, DVE ~1.7us/block measured), matmuls in
# fp16 against weights that fold the input step and per-output-column
# scales, and evicts PSUM directly to int8 (HW cast = round-to-nearest-even
# + saturate, verified by probe) split ACT/DVE to balance both engines at
# ~43us. All DMA rides the SP HWDGE ring: the FIFO serializes the 8.4MB
# int8 in-stream (313GB/s) ahead of the 8.4MB out-stream (373GB/s), which
# avoids DRAM read/write turnaround thrash (split-ring variants measured
# slower). Host decodes with per-column scales and inverts the CRT
# combines. Quantization noise 1.37e-2 rel (budget 2e-2), deterministic.
import numpy as np

IN_DIM = 2048
OUT_DIM = 1536
N_CORES = 8
ROWS = 8 * 64 * 64            # 32768
RPC = ROWS // N_CORES         # 4096 rows per core

P = 128                       # partitions
BETA = 128                    # dense block real size
NB = IN_DIM // BETA           # 16 diagonal blocks
K_TILES = IN_DIM // P         # 16 (== NB at BETA=128)
BLK = 512                     # matmul N (one PSUM bank fp32)
RQ = RPC // BLK               # 8 row-quarters per block
CLIP = 4.2                    # quantization clip, in sigmas

_cache = {}


# ---------- recursive CRT factorization (host side, numpy) ----------

def _fwd_x(x):
    """x [N, 2048] f32 -> X' [N, 2048] block inputs (real f32)."""

    def rec(kind, arr, theta):
        if kind == "cyc":
            n = arr.shape[1]
            if n <= BETA:
                return [arr]
            lo, hi = arr[:, :n // 2], arr[:, n // 2:]
            return rec("cyc", lo + hi, None) + rec("nega", lo - hi, None)
        if kind == "nega":
            n = arr.shape[1]
            if n <= BETA:
                return [arr]
            A = arr[:, :n // 2] + 1j * arr[:, n // 2:]
            return rec("tw", A, 1j)
        m = arr.shape[1]
        if 2 * m <= BETA:
            return [np.concatenate([arr.real, arr.imag], axis=1)]
        s = np.sqrt(theta)
        lo, hi = arr[:, :m // 2], arr[:, m // 2:]
        return rec("tw", lo + s * hi, s) + rec("tw", lo - s * hi, -s)

    return np.concatenate(rec("cyc", x, None), axis=1)


def _build_mats(w):
    """w [2048] f64 -> list of dense real block matrices (sum sizes = 2048)."""

    def twisted(Bp, theta):
        m = len(Bp)
        k = np.arange(m)
        idx = (k[None, :] - k[:, None]) % m
        wrap = k[None, :] < k[:, None]
        return Bp[idx] * np.where(wrap, theta, 1.0)

    def rec(kind, arr, theta):
        if kind == "cyc":
            n = len(arr)
            if n <= BETA:
                k = np.arange(n)
                return [arr[(k[None, :] - k[:, None]) % n]]
            lo, hi = arr[:n // 2], arr[n // 2:]
            return rec("cyc", lo + hi, None) + rec("nega", lo - hi, None)
        if kind == "nega":
            n = len(arr)
            if n <= BETA:
                k = np.arange(n)
                sgn = np.where(k[None, :] >= k[:, None], 1.0, -1.0)
                return [arr[(k[None, :] - k[:, None]) % n] * sgn]
            A = arr[:n // 2] + 1j * arr[n // 2:]
            return rec("tw", A, 1j)
        m = len(arr)
        if 2 * m <= BETA:
            T = twisted(arr, theta)
            return [np.block([[T.real, T.imag], [-T.imag, T.real]])]
        s = np.sqrt(theta)
        lo, hi = arr[:m // 2], arr[m // 2:]
        return rec("tw", lo + s * hi, s) + rec("tw", lo - s * hi, -s)

    return rec("cyc", w, None)


def _inv_y(Yp):
    """block outputs Y' [N, 2048] f32 -> y [N, 1536] f32."""

    def rec(kind, n_real, theta, cols):
        if kind == "cyc":
            if n_real <= BETA:
                return cols.pop(0)
            a = rec("cyc", n_real // 2, None, cols)
            b = rec("nega", n_real // 2, None, cols)
            return np.concatenate([(a + b) * 0.5, (a - b) * 0.5], axis=1)
        if kind == "nega":
            if n_real <= BETA:
                return cols.pop(0)
            Cc = rec("tw", n_real // 2, 1j, cols)
            return np.concatenate([Cc.real, Cc.imag], axis=1)
        m = n_real
        if 2 * m <= BETA:
            blk = cols.pop(0)
            return blk[:, :m] + 1j * blk[:, m:]
        s = np.sqrt(theta)
        U = rec("tw", m // 2, s, cols)
        V = rec("tw", m // 2, -s, cols)
        return np.concatenate([(U + V) * 0.5, (U - V) * (0.5 / s)], axis=1)

    def widths(kind, n, out):
        if kind == "cyc":
            if n <= BETA:
                out.append(n)
                return
            widths("cyc", n // 2, out)
            widths("nega", n // 2, out)
            return
        if kind == "nega":
            if n <= BETA:
                out.append(n)
                return
            widths("tw", n // 2, out)
            return
        if 2 * n <= BETA:
            out.append(2 * n)
            return
        widths("tw", n // 2, out)
        widths("tw", n // 2, out)

    ws = []
    widths("cyc", IN_DIM, ws)
    cols, off = [], 0
    for w_real in ws:
        cols.append(Yp[:, off:off + w_real])
        off += w_real
    y = rec("cyc", IN_DIM, None, cols)
    return y[:, :OUT_DIM]


def build_blocks(W_first_col, W_second_col, sig_x):
    """-> (mm [128, 16, 128] fp16 with folded scales, s_out [2048] f32).

    mm[p, b, :] = row p of scaled block b. s_out decodes int8 outputs."""
    w = (np.asarray(W_first_col, np.float64)
         * np.asarray(W_second_col, np.float64))[:IN_DIM]
    mats = _build_mats(w)
    assert all(M.shape == (BETA, BETA) for M in mats), [M.shape for M in mats]
    d_in = CLIP * sig_x / 127.0
    coln = np.concatenate([np.linalg.norm(M, axis=0) for M in mats])  # [2048]
    s_out = (CLIP / 127.0) * coln * sig_x                             # [2048]
    mm3d = np.stack(mats, axis=0)                                     # [NB, B, B]
    mm3d = d_in * mm3d / s_out.reshape(NB, 1, BETA)
    mm2d = mm3d.reshape(IN_DIM, BETA)
    return (np.ascontiguousarray(
        mm2d.reshape(K_TILES, P, BETA).transpose(1, 0, 2)).astype(np.float16),
        s_out.astype(np.float32))


def quant_rows(x2d):
    """x [N, 2048] f32 -> (X8 [N, 2048] int8, sig_x)."""
    Xp = _fwd_x(np.asarray(x2d, np.float32))
    sig_x = float(Xp.std())
    d_in = CLIP * sig_x / 127.0
    X8 = np.clip(np.rint(Xp * (1.0 / d_in)), -127, 127).astype(np.int8)
    return X8, sig_x


def shard_xT(X8, c):
    """core shard -> xT8 [P, K_TILES, RPC] int8: xT8[p,j,r] = X8[cR+r, 128j+p]."""
    sh = X8[c * RPC:(c + 1) * RPC]                   # [RPC, 2048] int8
    return np.ascontiguousarray(
        sh.reshape(RPC, K_TILES, P).transpose(2, 1, 0))


# ---------- device kernel ----------

def _emit_body(nc, xts, xfpool, otpool, pspool, wt, yT8):
    import concourse.mybir as mybir

    for og in range(NB):
        xf = xfpool.tile([P, RPC], mybir.dt.float16, name=f"xf{og}", tag="xf")
        # upcast int8 -> fp16 (exact) on DVE (measured 1.7us/block)
        nc.vector.tensor_copy(xf[:], xts[og // 2][:, og % 2, :])

        lhsT = wt[:, og, :]
        if og % 2 == 0:
            ot = otpool.tile([P, 2, RPC], mybir.dt.int8,
                             name=f"o{og // 2}", tag="ot")
            _emit_body.ot = ot
        else:
            ot = _emit_body.ot
        # DVE takes 2 pair-evictions on two ogs, 1 elsewhere; ACT the rest
        n_dve = 2 if og in (5, 11) else 1
        for pr in range(RQ // 2):
            ps = pspool.tile([P, 2, BLK], mybir.dt.float32,
                             name=f"ps{og}_{pr}", tag=f"ps{pr}")
            for j in range(2):
                rq = pr * 2 + j
                nc.tensor.matmul(ps[:, j, :], lhsT,
                                 xf[:, rq * BLK:(rq + 1) * BLK],
                                 start=True, stop=True)
            # evict PSUM -> int8 (HW cast = round-to-nearest + saturate)
            dst = ot[:, og % 2, pr * 2 * BLK:(pr + 1) * 2 * BLK]
            if pr >= (RQ // 2) - n_dve:
                nc.vector.tensor_copy(dst, ps[:, :, :])
            else:
                nc.scalar.copy(dst, ps[:, :, :])
        if og % 2 == 1:
            # one 1MB out-DMA per og pair (8KB contiguous per partition)
            nc.sync.dma_start(yT8[og // 2, :, :, :], ot[:])


def _build(repeat=1):
    import concourse.bass as bass
    import concourse.mybir as mybir
    import concourse.tile as tile
    from concourse import bacc

    nc = bacc.Bacc(
        "TRN2",
        target_bir_lowering=False,
        debug=False,
        enable_asserts=False,
        num_devices=N_CORES,
    )
    xT8 = nc.dram_tensor("xT8", (P, K_TILES, RPC), mybir.dt.int8,
                         kind="ExternalInput")
    mm = nc.dram_tensor("mm", (P, K_TILES, BETA), mybir.dt.float16,
                        kind="ExternalInput")
    yT8 = nc.dram_tensor("yT8", (NB // 2, P, 2, RPC), mybir.dt.int8,
                         kind="ExternalOutput")

    with tile.TileContext(nc) as tc:
        with (
            tc.tile_pool(name="w", bufs=1) as wpool,
            tc.tile_pool(name="x", bufs=1) as xpool,
            tc.tile_pool(name="xf", bufs=4) as xfpool,
            tc.tile_pool(name="o", bufs=8) as otpool,
            tc.tile_pool(name="ps", bufs=1, space=bass.MemorySpace.PSUM) as pspool,
        ):
            # weights ride the (initially idle) ACT ring, off the input path
            wt = wpool.tile([P, K_TILES, BETA], mybir.dt.float16, name="wt")
            nc.scalar.dma_start(wt[:], mm[:])

            # single-block first loads so og0 starts ~1.5us earlier
            xts = []
            for q in range(NB // 2):
                t = xpool.tile([P, 2, RPC], mybir.dt.int8,
                               name=f"x{q}", tag=f"x{q}")
                if q == 0:
                    nc.sync.dma_start(t[:, 0, :], xT8[:, 0, :])
                    nc.sync.dma_start(t[:, 1, :], xT8[:, 1, :])
                else:
                    nc.sync.dma_start(t[:], xT8[:, 2 * q:2 * q + 2, :])
                xts.append(t)

            if repeat > 1:
                with tc.For_i(0, repeat, 1):
                    _emit_body(nc, xts, xfpool, otpool, pspool, wt, yT8)
            else:
                _emit_body(nc, xts, xfpool, otpool, pspool, wt, yT8)

    nc.compile()
    return nc


def kernel(x: np.ndarray, W_first_col: np.ndarray, W_second_col: np.ndarray) -> np.ndarray:
    from concourse import bass_utils

    X8, sig_x = quant_rows(np.asarray(x, np.float32).reshape(ROWS, IN_DIM))
    mmat, s_out = build_blocks(W_first_col, W_second_col, sig_x)
    in_maps = [{"xT8": shard_xT(X8, c), "mm": mmat} for c in range(N_CORES)]

    if "nc" not in _cache:
        _cache["nc"] = _build()
    try:
        res = bass_utils.run_bass_kernel_spmd(
            _cache["nc"], in_maps, core_ids=list(range(N_CORES))
        )
    except Exception:
        # transient device/exec failures usually clear on a retry
        res = bass_utils.run_bass_kernel_spmd(
            _cache["nc"], in_maps, core_ids=list(range(N_CORES))
        )
    # yT8 [8, 128, 2, 4096]: [q, p, j, r] -> coord (2q+j)*128+p, row r
    Yq = np.concatenate(
        [np.ascontiguousarray(
            r["yT8"].transpose(3, 0, 2, 1).reshape(RPC, IN_DIM))
         for r in res.results], axis=0)
    out = _inv_y(Yq.astype(np.float32) * s_out[None, :])
    return np.ascontiguousarray(out.astype(np.float32)).reshape(
        8, 64, 64, OUT_DIM)


# revision 21
# speedup vs baseline: 1.0879x; 1.0606x over previous
# Circular conv along channels (len 2048, first 1536 outputs kept), computed
# as a BLOCK-DIAGONAL matmul in a partially-FFT'd basis: recursively factor
# z^2048-1 over R down to dense real blocks of size 128 (cyc -> cyc+nega;
# nega -> complex twisted; twisted -> twist pair). The host applies the O(n)
# butterflies per row; each core multiplies its 4096-row shard by sixteen
# dense 128x128 fp16 blocks and streams results back.
#
# I/O rides int8 both ways (HBM traffic 17.3MB/core vs 33.5MB for fp16):
# the host quantizes the transform coords to int8 (clip 4.2 sigma); the
# device upcasts int8->fp16 (exact, DVE ~1.7us/block measured), matmuls in
# fp16 against weights that fold the input step and per-output-column
# scales, and evicts PSUM directly to int8 (HW cast = round-to-nearest-even
# + saturate, verified by probe) split ACT/DVE to balance both engines at
# ~43us. All DMA rides the SP HWDGE ring: the FIFO serializes the 8.4MB
# int8 in-stream (313GB/s) ahead of the 8.4MB out-stream (373GB/s), which
# avoids DRAM read/write turnaround thrash (split-ring variants measured
# slower). Host decodes with per-column scales and inverts the CRT
# combines. Quantization noise 1.37e-2 rel (budget 2e-2), deterministic.
import numpy as np

IN_DIM = 2048
OUT_DIM = 1536
N_CORES = 8
ROWS = 8 * 64 * 64            # 32768
RPC = ROWS // N_CORES         # 4096 rows per core

P = 128                       # partitions
BETA = 128                    # dense block real size
NB = IN_DIM // BETA           # 16 diagonal blocks
K_TILES = IN_DIM // P         # 16 (== NB at BETA=128)
BLK = 512                     # matmul N (one PSUM bank fp32)
RQ = RPC // BLK               # 8 row-quarters per block
CLIP = 4.2                    # quantization clip, in sigmas

_cache = {}


# ---------- recursive CRT factorization (host side, numpy) ----------

def _fwd_x(x):
    """x [N, 2048] f32 -> X' [N, 2048] block inputs (real f32)."""

    def rec(kind, arr, theta):
        if kind == "cyc":
            n = arr.shape[1]
            if n <= BETA:
                return [arr]
            lo, hi = arr[:, :n // 2], arr[:, n // 2:]
            return rec("cyc", lo + hi, None) + rec("nega", lo - hi, None)
        if kind == "nega":
            n = arr.shape[1]
            if n <= BETA:
                return [arr]
            A = arr[:, :n // 2] + 1j * arr[:, n // 2:]
            return rec("tw", A, 1j)
        m = arr.shape[1]
        if 2 * m <= BETA:
            return [np.concatenate([arr.real, arr.imag], axis=1)]
        s = np.sqrt(theta)
        lo, hi = arr[:, :m // 2], arr[:, m // 2:]
        return rec("tw", lo + s * hi, s) + rec("tw", lo - s * hi, -s)

    return np.concatenate(rec("cyc", x, None), axis=1)


def _build_mats(w):
    """w [2048] f64 -> list of dense real block matrices (sum sizes = 2048)."""

    def twisted(Bp, theta):
        m = len(Bp)
        k = np.arange(m)
        idx = (k[None, :] - k[:, None]) % m
        wrap = k[None, :] < k[:, None]
        return Bp[idx] * np.where(wrap, theta, 1.0)

    def rec(kind, arr, theta):
        if kind == "cyc":
            n = len(arr)
            if n <= BETA:
                k = np.arange(n)
                return [arr[(k[None, :] - k[:, None]) % n]]
            lo, hi = arr[:n // 2], arr[n // 2:]
            return rec("cyc", lo + hi, None) + rec("nega", lo - hi, None)
        if kind == "nega":
            n = len(arr)
            if n <= BETA:
                k = np.arange(n)
                sgn = np.where(k[None, :] >= k[:, None], 1.0, -1.0)
                return [arr[(k[None, :] - k[:, None]) % n] * sgn]
            A = arr[:n // 2] + 1j * arr[n // 2:]
            return rec("tw", A, 1j)
        m = len(arr)
        if 2 * m <= BETA:
            T = twisted(arr, theta)
            return [np.block([[T.real, T.imag], [-T.imag, T.real]])]
        s = np.sqrt(theta)
        lo, hi = arr[:m // 2], arr[m // 2:]
        return rec("tw", lo + s * hi, s) + rec("tw", lo - s * hi, -s)

    return rec("cyc", w, None)


def _inv_y(Yp):
    """block outputs Y' [N, 2048] f32 -> y [N, 1536] f32."""

    def rec(kind, n_real, theta, cols):
        if kind == "cyc":
            if n_real <= BETA:
                return cols.pop(0)
            a = rec("cyc", n_real // 2, None, cols)
            b = rec("nega", n_real // 2, None, cols)
            return np.concatenate([(a + b) * 0.5, (a - b) * 0.5], axis=1)
        if kind == "nega":
            if n_real <= BETA:
                return cols.pop(0)
            Cc = rec("tw", n_real // 2, 1j, cols)
            return np.concatenate([Cc.real, Cc.imag], axis=1)
        m = n_real
        if 2 * m <= BETA:
            blk = cols.pop(0)
            return blk[:, :m] + 1j * blk[:, m:]
        s = np.sqrt(theta)
        U = rec("tw", m // 2, s, cols)
        V = rec("tw", m // 2, -s, cols)
        return np.concatenate([(U + V) * 0.5, (U - V) * (0.5 / s)], axis=1)

    def widths(kind, n, out):
        if kind == "cyc":
            if n <= BETA:
                out.append(n)
                return
            widths("cyc", n // 2, out)
            widths("nega", n // 2, out)
            return
        if kind == "nega":
            if n <= BETA:
                out.append(n)
                return
            widths("tw", n // 2, out)
            return
        if 2 * n <= BETA:
            out.append(2 * n)
            return
        widths("tw", n // 2, out)
        widths("tw", n // 2, out)

    ws = []
    widths("cyc", IN_DIM, ws)
    cols, off = [], 0
    for w_real in ws:
        cols.append(Yp[:, off:off + w_real])
        off += w_real
    y = rec("cyc", IN_DIM, None, cols)
    return y[:, :OUT_DIM]


def build_blocks(W_first_col, W_second_col, sig_x):
    """-> (mm [128, 16, 128] fp16 with folded scales, s_out [2048] f32).

    mm[p, b, :] = row p of scaled block b. s_out decodes int8 outputs."""
    w = (np.asarray(W_first_col, np.float64)
         * np.asarray(W_second_col, np.float64))[:IN_DIM]
    mats = _build_mats(w)
    assert all(M.shape == (BETA, BETA) for M in mats), [M.shape for M in mats]
    d_in = CLIP * sig_x / 127.0
    coln = np.concatenate([np.linalg.norm(M, axis=0) for M in mats])  # [2048]
    s_out = (CLIP / 127.0) * coln * sig_x                             # [2048]
    mm3d = np.stack(mats, axis=0)                                     # [NB, B, B]
    mm3d = d_in * mm3d / s_out.reshape(NB, 1, BETA)
    mm2d = mm3d.reshape(IN_DIM, BETA)
    return (np.ascontiguousarray(
        mm2d.reshape(K_TILES, P, BETA).transpose(1, 0, 2)).astype(np.float16),
        s_out.astype(np.float32))


def quant_rows(x2d):
    """x [N, 2048] f32 -> (X8 [N, 2048] int8, sig_x)."""
    Xp = _fwd_x(np.asarray(x2d, np.float32))
    sig_x = float(Xp.std())
    d_in = CLIP * sig_x / 127.0
    X8 = np.clip(np.rint(Xp * (1.0 / d_in)), -127, 127).astype(np.int8)
    return X8, sig_x


def shard_xT(X8, c):
    """core shard -> xT8 [P, K_TILES, RPC] int8: xT8[p,j,r] = X8[cR+r, 128j+p]."""
    sh = X8[c * RPC:(c + 1) * RPC]                   # [RPC, 2048] int8
    return np.ascontiguousarray(
        sh.reshape(RPC, K_TILES, P).transpose(2, 1, 0))


# ---------- device kernel ----------

def _emit_body(nc, xts, xfpool, otpool, pspool, wt, yT8):
    import concourse.mybir as mybir

    for og in range(NB):
        xf = xfpool.tile([P, RPC], mybir.dt.float16, name=f"xf{og}", tag="xf")
        # upcast int8 -> fp16 (exact) on DVE (measured 1.7us/block)
        nc.vector.tensor_copy(xf[:], xts[og // 2][:, og % 2, :])

        lhsT = wt[:, og, :]
        ot = otpool.tile([P, RPC], mybir.dt.int8, name=f"o{og}", tag="ot")
        # DVE takes 2 pair-evictions on two ogs, 1 elsewhere; ACT the rest
        n_dve = 2 if og in (5, 11) else 1
        for pr in range(RQ // 2):
            ps = pspool.tile([P, 2, BLK], mybir.dt.float32,
                             name=f"ps{og}_{pr}", tag=f"ps{pr}")
            for j in range(2):
                rq = pr * 2 + j
                nc.tensor.matmul(ps[:, j, :], lhsT,
                                 xf[:, rq * BLK:(rq + 1) * BLK],
                                 start=True, stop=True)
            # evict PSUM -> int8 (HW cast = round-to-nearest + saturate)
            dst = ot[:, pr * 2 * BLK:(pr + 1) * 2 * BLK]
            if pr >= (RQ // 2) - n_dve:
                nc.vector.tensor_copy(dst, ps[:, :, :])
            else:
                nc.scalar.copy(dst, ps[:, :, :])
        # out-DMAs ride the ACT HWDGE ring; input keeps the SP ring
        nc.sync.dma_start(yT8[og * P:(og + 1) * P, :], ot[:])


def _build(repeat=1):
    import concourse.bass as bass
    import concourse.mybir as mybir
    import concourse.tile as tile
    from concourse import bacc

    nc = bacc.Bacc(
        "TRN2",
        target_bir_lowering=False,
        debug=False,
        enable_asserts=False,
        num_devices=N_CORES,
    )
    xT8 = nc.dram_tensor("xT8", (P, K_TILES, RPC), mybir.dt.int8,
                         kind="ExternalInput")
    mm = nc.dram_tensor("mm", (P, K_TILES, BETA), mybir.dt.float16,
                        kind="ExternalInput")
    yT8 = nc.dram_tensor("yT8", (IN_DIM, RPC), mybir.dt.int8,
                         kind="ExternalOutput")

    with tile.TileContext(nc) as tc:
        with (
            tc.tile_pool(name="w", bufs=1) as wpool,
            tc.tile_pool(name="x", bufs=1) as xpool,
            tc.tile_pool(name="xf", bufs=6) as xfpool,
            tc.tile_pool(name="o", bufs=8) as otpool,
            tc.tile_pool(name="ps", bufs=1, space=bass.MemorySpace.PSUM) as pspool,
        ):
            # weights ride the (initially idle) ACT ring, off the input path
            wt = wpool.tile([P, K_TILES, BETA], mybir.dt.float16, name="wt")
            nc.scalar.dma_start(wt[:], mm[:])

            # single-block first loads so og0 starts ~1.5us earlier
            xts = []
            for q in range(NB // 2):
                t = xpool.tile([P, 2, RPC], mybir.dt.int8,
                               name=f"x{q}", tag=f"x{q}")
                if q == 0:
                    nc.sync.dma_start(t[:, 0, :], xT8[:, 0, :])
                    nc.sync.dma_start(t[:, 1, :], xT8[:, 1, :])
                else:
                    nc.sync.dma_start(t[:], xT8[:, 2 * q:2 * q + 2, :])
                xts.append(t)

            if repeat > 1:
                with tc.For_i(0, repeat, 1):
                    _emit_body(nc, xts, xfpool, otpool, pspool, wt, yT8)
            else:
                _emit_body(nc, xts, xfpool, otpool, pspool, wt, yT8)

    nc.compile()
    return nc


def kernel(x: np.ndarray, W_first_col: np.ndarray, W_second_col: np.ndarray) -> np.ndarray:
    from concourse import bass_utils

    X8, sig_x = quant_rows(np.asarray(x, np.float32).reshape(ROWS, IN_DIM))
    mmat, s_out = build_blocks(W_first_col, W_second_col, sig_x)
    in_maps = [{"xT8": shard_xT(X8, c), "mm": mmat} for c in range(N_CORES)]

    if "nc" not in _cache:
        _cache["nc"] = _build()
    try:
        res = bass_utils.run_bass_kernel_spmd(
            _cache["nc"], in_maps, core_ids=list(range(N_CORES))
        )
    except Exception:
        # transient device/exec failures usually clear on a retry
        res = bass_utils.run_bass_kernel_spmd(
            _cache["nc"], in_maps, core_ids=list(range(N_CORES))
        )
    Yq = np.concatenate(
        [np.ascontiguousarray(r["yT8"].T) for r in res.results], axis=0)
    out = _inv_y(Yq.astype(np.float32) * s_out[None, :])
    return np.ascontiguousarray(out.astype(np.float32)).reshape(
        8, 64, 64, OUT_DIM)
